# revision 53
# baseline (speedup 1.0000x reference)
"""Trainium2 Bass kernel for a GQA attention block (B=2, S=2048, H=2048,
16 q-heads / 8 kv-heads, head_dim=128, fp32), tensor-parallel over heads
across 8 NeuronCores.

Per-core shard (core c): q-heads {2c, 2c+1}, kv-head c; wq/wk/wv column
shards, wo row shard. x is replicated (pre-transposed on host so the
contraction dim lands on SBUF partitions). Each core emits a bf16 partial
[4096, 2048] o-proj product; the host gathers by summing the 8 partials
in fp32.

All matmul operands are bf16 (PSUM accumulation and softmax statistics in
fp32); rel-err lands ~4-6e-3 against the fp32 reference, well inside the
2e-2 gate.

Device dataflow (per core):
  A) QKV^T projections ([d, tok] layout): per token-tile one batched x DMA
     ([128, 16, 512] in 4 chunks), two 2-bank PSUM slabs ((q0,q1) and
     (k,v)); per q/k head-slab: ACT evicts PSUM->bf16, RMSNorm
     sum-of-squares on Pool (tensor-mul + partition all-reduce), sqrt on
     ACT, reciprocal on DVE; RoPE rotate-half as a signed-permutation
     matmul; the rstd scale applied last (commutes -- column-uniform).
     V^T is evicted to bf16 and transposed to natural [tok, d] via DMA
     XBAR transpose (no PE/ACT involvement).
  B) Causal attention, per (batch, 512-q-tile, head):
     scores S^T [128 k, 512 q] = K^T_tile.T @ Q^T, exp on ACT (no max
     subtraction -- RMSNorm bounds |scores| <= sqrt(128)), causal masking
     by DVE multiply with 4 precomputed diagonal band masks; softmax
     denominator via DVE bf16 tile-sum tree + Pool partition all-reduce
     (no PE ones-matmuls); PV accumulated over k-tiles. The row-parallel
     o-proj for each q-tile is emitted one iteration later so the
     denominator latency hides under the next tile's score matmuls.
"""

import math
import os
import sys

import numpy as np

for _p in ("/opt/trn_rl_repo", "/root/.axon_site/_ro/trn_rl_repo"):
    if os.path.isdir(_p) and _p not in sys.path:
        sys.path.insert(0, _p)
        break

import concourse.bacc as bacc
import concourse.tile as tile
from concourse import mybir
from concourse.bass_isa import ReduceOp
from concourse.bass_utils import run_bass_kernel_spmd

# Problem constants (hardcoded per contract)
B, S, HID = 2, 2048, 2048
NH, NKV, D = 16, 8, 128
NCORES = 8
HQ = NH // NCORES  # q heads per core = 2
T = B * S          # 4096 tokens
EPS = 1e-5
F32 = mybir.dt.float32
BF16 = mybir.dt.bfloat16
SCALE = 1.0 / math.sqrt(D)

KT = HID // 128      # 16 contraction tiles
QT_PER_B = S // 512  # 4 q-tiles of 512 per batch


def build_nc():
    nc = bacc.Bacc("TRN2", target_bir_lowering=False, debug=False)
    xt = nc.dram_tensor("xt", [HID, T], BF16, kind="ExternalInput").ap()
    wqkv = nc.dram_tensor("wqkv", [HID, 4 * D], BF16, kind="ExternalInput").ap()
    woc = nc.dram_tensor("woc", [HQ * D, HID], BF16, kind="ExternalInput").ap()
    pmat = nc.dram_tensor("pmat", [D, D], BF16, kind="ExternalInput").ap()
    tabs = nc.dram_tensor("tabs", [D, 4, S], BF16, kind="ExternalInput").ap()
    masks = nc.dram_tensor("masks", [D, 4, 512], BF16, kind="ExternalInput").ap()
    out = nc.dram_tensor("out", [T, HID], BF16, kind="ExternalOutput").ap()

    xt4 = xt.rearrange("(k p) t -> p k t", p=128)      # [128, 16, T]
    wqkv4 = wqkv.rearrange("(k p) m -> p k m", p=128)  # [128, 16, 512]

    with tile.TileContext(nc) as tc:
        from contextlib import ExitStack

        with ExitStack() as root:
            const = root.enter_context(tc.tile_pool(name="const", bufs=1))
            pmat_sb = const.tile([D, D], BF16, name="pmat_sb")
            eps_col = const.tile([128, 1], F32, name="eps_col")
            nc.vector.memset(eps_col, EPS)
            mask_sb = const.tile([128, 4, 512], BF16, name="mask_sb")

            res = root.enter_context(tc.tile_pool(name="res", bufs=1))
            wo_sb = res.tile([128, HQ, HID], BF16, name="wo_sb")
            qt_sb = res.tile([128, HQ, T], BF16, name="qt_sb")   # [d, h, tok]
            kt_sb = res.tile([128, T], BF16, name="kt_sb")       # [d, tok]
            v_sb = res.tile([128, T // 128, D], BF16, name="v_sb")  # [tok%128, tile, d]

            # ---------------- Phase A: QKV^T, norm, rope, V transpose ---------
            # SBUF pools stay open into phase B (the last tile's norm chains
            # are emitted after phase B's first scores, see defer below); the
            # PSUM pools close right after the token-tile loop
            pa = root.enter_context(ExitStack())
            if True:
                tabp = pa.enter_context(tc.tile_pool(name="tabp", bufs=2))
                wp = pa.enter_context(tc.tile_pool(name="wp", bufs=2))
                paPS = pa.enter_context(ExitStack())
                wqp = paPS.enter_context(tc.tile_pool(name="wqp", bufs=1))
                xp = paPS.enter_context(tc.tile_pool(name="xp", bufs=3))
                psA = paPS.enter_context(tc.tile_pool(name="psA", bufs=2, space="PSUM"))
                psR = paPS.enter_context(tc.tile_pool(name="psR", bufs=3, space="PSUM"))
                deferred = []

                wqkv_sb = wqp.tile([128, KT, 4 * D], BF16, name="wqkv_sb")
                nc.scalar.dma_start(out=pmat_sb, in_=pmat)

                # visit token tiles as (b0, b1) pairs sharing a sequence
                # position so each RoPE table slice is fetched once
                tab = None
                for ti, t in enumerate((0, 4, 1, 5, 2, 6, 3, 7)):
                    xk = xp.tile([128, KT, 512], BF16, name="xk", tag="xk")
                    if ti == 0:
                        # fine-grained interleave so the first matmul can
                        # start after one small (w, x) chunk pair
                        k0 = 0
                        for w in (1, 1, 2, 4, 4, 4):
                            nc.sync.dma_start(
                                out=wqkv_sb[:, k0:k0 + w, :],
                                in_=wqkv4[:, k0:k0 + w, :],
                            )
                            nc.sync.dma_start(
                                out=xk[:, k0:k0 + w, :],
                                in_=xt4[:, k0:k0 + w, t * 512:(t + 1) * 512],
                            )
                            k0 += w
                    else:
                        for c in range(4):
                            nc.sync.dma_start(
                                out=xk[:, 4 * c:4 * c + 4, :],
                                in_=xt4[:, 4 * c:4 * c + 4, t * 512:(t + 1) * 512],
                            )
                    if ti == 1:
                        nc.sync.dma_start(out=mask_sb, in_=masks)
                    if ti == 5:  # wo is not needed until phase B
                        nc.sync.dma_start(
                            out=wo_sb, in_=woc.rearrange("(h p) n -> p h n", p=128)
                        )
                    s0 = (t % QT_PER_B) * 512  # position-in-sequence of this tile
                    if ti % 2 == 0:  # second tile of each pair reuses the slice
                        tab = tabp.tile([128, 4, 512], BF16, name="tab", tag="tab")
                        nc.sync.dma_start(out=tab, in_=tabs[:, :, s0:s0 + 512])
                    # two 2-bank PSUM slabs: (q0,q1) and (k,v)
                    slabs = []
                    for g in range(2):
                        ps = psA.tile([128, 2, 512], F32, name="ps_qkv", tag="ps_qkv")
                        for k in range(KT):
                            for mm in range(2):
                                m = g * 2 + mm
                                nc.tensor.matmul(
                                    ps[:, mm, :],
                                    lhsT=(wqkv_sb[:, k, m * 128:(m + 1) * 128]),
                                    rhs=(xk[:, k, :]),
                                    start=(k == 0),
                                    stop=(k == KT - 1),
                                )
                        slabs.append(ps)

                    # evict all four PSUM slices first (ACT back-to-back) so
                    # the slabs free as early as possible, then run the
                    # rotate-half shuffles + their evictions so all PSUM use
                    # finishes before the (slow) norm chains start
                    qks = []
                    for m in range(3):
                        qk = wp.tile([128, 512], BF16, name="qk", tag=f"qk{m}")
                        nc.scalar.copy(qk, slabs[m // 2][:, m % 2, :])
                        qks.append(qk)
                    vt = wp.tile([128, 512], BF16, name="vt", tag="vt")
                    nc.scalar.copy(vt, slabs[1][:, 1, :])
                    for j in range(4):
                        nc.sync.dma_start_transpose(
                            out=v_sb[:, t * 4 + j, :],
                            in_=vt[:, j * 128:(j + 1) * 128],
                        )
                    shbs = []
                    for m in range(3):
                        shf = psR.tile([128, 512], F32, name="shf", tag="shf")
                        nc.tensor.matmul(shf, lhsT=pmat_sb, rhs=qks[m], start=True, stop=True)
                        shb = wp.tile([128, 512], BF16, name="shb", tag=f"shb{m}")
                        nc.scalar.copy(shb, shf)
                        shbs.append(shb)
                    def norm_rope_chains(t, tab, qks, shbs):
                        for m, ci, si in ((0, 0, 1), (1, 0, 1), (2, 2, 3)):
                            cosT = tab[:, ci, :]
                            sinT = tab[:, si, :]
                            qk = qks[m]
                            sq = wp.tile([128, 512], F32, name="sq", tag="sq")
                            nc.vector.tensor_mul(sq, qk, qk)
                            # row 0 holds the partition sum after the reduce
                            nc.gpsimd.partition_all_reduce(sq, sq, 128, ReduceOp.add)
                            rrow = wp.tile([1, 512], F32, name="rrow", tag="rrow")
                            nc.scalar.activation(
                                rrow, sq[0:1, :], mybir.ActivationFunctionType.Sqrt,
                                bias=eps_col[0:1, :], scale=1.0 / D,
                            )
                            brow = wp.tile([1, 512], BF16, name="brow", tag="brow")
                            with nc.allow_low_precision(reason="bf16 rstd, matches operand precision"):
                                nc.vector.reciprocal(brow, rrow)
                            rstd = wp.tile([128, 512], BF16, name="rstd", tag="rstd")
                            nc.gpsimd.partition_broadcast(rstd, brow)
                            # all-bf16 SBUF operands keep DVE in its 4x mode
                            t0 = wp.tile([128, 512], BF16, name="t0", tag="t0")
                            nc.vector.tensor_mul(t0, qk, cosT)
                            t1 = wp.tile([128, 512], BF16, name="t1", tag="t1")
                            nc.vector.tensor_mul(t1, shbs[m], sinT)
                            tr = wp.tile([128, 512], BF16, name="tr", tag="tr")
                            nc.vector.tensor_add(tr, t0, t1)
                            if m < 2:
                                dst = qt_sb[:, m, t * 512:(t + 1) * 512]
                            else:
                                dst = kt_sb[:, t * 512:(t + 1) * 512]
                            nc.vector.tensor_mul(dst, tr, rstd)

                    norm_rope_chains(t, tab, qks, shbs)

            paPS.close()  # free phase-A PSUM banks for the attention pools

            # ---------------- Phase B: causal attention + o-proj --------------
            with ExitStack() as pb:
                ep = pb.enter_context(tc.tile_pool(name="ep", bufs=18))
                dp = pb.enter_context(tc.tile_pool(name="dp", bufs=3))
                atp = pb.enter_context(tc.tile_pool(name="atp", bufs=6))
                op = pb.enter_context(tc.tile_pool(name="op", bufs=4))
                psS = pb.enter_context(tc.tile_pool(name="psS", bufs=2, space="PSUM"))
                psO = pb.enter_context(tc.tile_pool(name="psO", bufs=2, space="PSUM"))
                psP = pb.enter_context(tc.tile_pool(name="psP", bufs=2, space="PSUM"))

                def emit_oproj(b, q0, ats, final=False):
                    # row-parallel o-proj partial for rows [b*S+q0, +512)
                    for mq in range(4):
                        ob = op.tile([128, HID], BF16, name="ob", tag="ob")
                        for nn in range(4):
                            po = psP.tile([128, 512], F32, name="po", tag="po")
                            for h in range(HQ):
                                nc.tensor.matmul(
                                    po,
                                    lhsT=(ats[h][:, mq * 128:(mq + 1) * 128]),
                                    rhs=(wo_sb[:, h, nn * 512:(nn + 1) * 512]),
                                    start=(h == 0), stop=(h == HQ - 1),
                                )
                            dst = ob[:, nn * 512:(nn + 1) * 512]
                            if nn % 2 == 0:
                                nc.scalar.copy(dst, po)
                            else:
                                nc.vector.tensor_copy(dst, po)
                            if final and mq == 3:
                                # stream the tail out per column chunk so the
                                # drain after the last matmul is one chunk
                                nc.sync.dma_start(
                                    out=out[b * S + q0 + mq * 128: b * S + q0 + (mq + 1) * 128,
                                            nn * 512:(nn + 1) * 512],
                                    in_=dst,
                                )
                        if not (final and mq == 3):
                            nc.sync.dma_start(
                                out=out[b * S + q0 + mq * 128: b * S + q0 + (mq + 1) * 128, :],
                                in_=ob,
                            )

                pend = None
                # qt3 first: its long score run covers the phase-boundary ACT
                # drain (last tile's eviction chain + the exp table load)
                for b in range(B):
                    for qt in (3, 0, 1, 2):
                        q0 = qt * 512
                        n_kt = 4 * (qt + 1)  # causal k tiles for this q-tile
                        # pass 1: scores + exp + causal mask for both heads
                        ets = {}
                        for h in range(HQ):
                            ets[h] = []
                            for kp in range(n_kt // 2):
                                st = psS.tile([128, 2, 512], F32, name="st", tag="st")
                                for j in range(2):
                                    kt = 2 * kp + j
                                    nc.tensor.matmul(
                                        st[:, j, :],
                                        lhsT=(kt_sb[:, b * S + kt * 128: b * S + (kt + 1) * 128]),
                                        rhs=(qt_sb[:, h, b * S + q0: b * S + q0 + 512]),
                                        start=True, stop=True,
                                    )
                                etp = ep.tile([128, 2, 512], BF16, name="et", tag="et")
                                nc.scalar.activation(
                                    etp, st, mybir.ActivationFunctionType.Exp,
                                    scale=SCALE,
                                )
                                for j in range(2):
                                    kt = 2 * kp + j
                                    et = etp[:, j, :]
                                    o = kt * 128 - q0
                                    if o >= 0:  # diagonal band: mask via DVE
                                        nc.vector.tensor_mul(
                                            et, et, mask_sb[:, o // 128, :]
                                        )
                                    ets[h].append(et)
                        for d_args in deferred:  # last tile's norm chains
                            norm_rope_chains(*d_args)
                        deferred = []
                        # pass 2: denominator (DVE sums + Pool all-reduce,
                        # no PE work), PV accumulation, normalization
                        ats = {}
                        for h in range(HQ):
                            eh = ets[h]
                            acc = dp.tile([128, 512], BF16, name="dacc", tag="dacc")
                            nc.vector.tensor_add(acc, eh[0], eh[1])
                            for kt in range(2, n_kt):
                                nc.vector.tensor_add(acc, acc, eh[kt])
                            den = dp.tile([128, 512], F32, name="den", tag="den")
                            nc.gpsimd.partition_all_reduce(den, acc, 128, ReduceOp.add)
                            rd = dp.tile([1, 512], BF16, name="rd", tag="rd")
                            with nc.allow_low_precision(reason="bf16 softmax scale, matches operand precision"):
                                nc.vector.reciprocal(rd, den[0:1, :])
                            rb = dp.tile([128, 512], BF16, name="rb", tag="rb")
                            nc.gpsimd.partition_broadcast(rb, rd)
                            ot = psO.tile([128, 512], F32, name="ot", tag="ot")
                            for kt in range(n_kt):
                                nc.tensor.matmul(
                                    ot, lhsT=(v_sb[:, b * (S // 128) + kt, :]),
                                    rhs=(eh[kt]),
                                    start=(kt == 0), stop=(kt == n_kt - 1),
                                )
                            at = atp.tile([128, 512], BF16, name="at", tag="at")
                            nc.vector.tensor_mul(at, ot, rb)
                            ats[h] = at
                        # o-proj of the PREVIOUS q-tile lands here, hiding
                        # this tile's denominator latency under PE work
                        if pend is not None:
                            emit_oproj(*pend)
                        pend = (b, q0, ats)
                emit_oproj(*pend, final=True)
    nc.compile()
    return nc


def _rot_half(w):
    return np.concatenate([w[D // 2:], w[:D // 2]])


def prep_inputs(x, cos, sin, wq, wk, wv, wo, q_norm_w, k_norm_w):
    """Host-side sharding/layout prep. Returns per-core in_maps."""
    import ml_dtypes
    f = np.float32
    mf = np.dtype(ml_dtypes.bfloat16)
    cvt = lambda a: np.ascontiguousarray(np.asarray(a, f).astype(mf))
    x = np.asarray(x, f)
    cos = np.asarray(cos, f)
    sin = np.asarray(sin, f)
    wq, wk, wv, wo = (np.asarray(a, f) for a in (wq, wk, wv, wo))
    q_norm_w = np.asarray(q_norm_w, f)
    k_norm_w = np.asarray(k_norm_w, f)

    xt = np.ascontiguousarray(x.reshape(T, HID).T)  # [HID, T]
    # RoPE tables with the q/k norm weights folded in: (cq, sq, ck, sk)
    tabs = np.stack([
        cos.T * q_norm_w[:, None],
        sin.T * _rot_half(q_norm_w)[:, None],
        cos.T * k_norm_w[:, None],
        sin.T * _rot_half(k_norm_w)[:, None],
    ], axis=1)  # [D, 4, S]
    # rotate-half permutation (with sign) as a matmul stationary operand:
    # out[d] = sum_j pmat[j, d] * q[j] = sign(d) * q[(d+64) % 128]
    pmat = np.zeros((D, D), f)
    for d in range(D // 2):
        pmat[d + D // 2, d] = -1.0
    for d in range(D // 2, D):
        pmat[d - D // 2, d] = 1.0
    # causal diagonal band masks, one per k-tile offset within a 512-q tile
    kr = np.arange(128)[:, None]
    qc = np.arange(512)[None, :]
    masks = np.stack(
        [(kr + o <= qc).astype(f) for o in (0, 128, 256, 384)], axis=1
    )  # [128, 4, 512]
    xt_m, tabs_m, pmat_m, masks_m = (cvt(a) for a in (xt, tabs, pmat, masks))

    in_maps = []
    for c in range(NCORES):
        wqkv_c = np.ascontiguousarray(np.concatenate([
            wq[:, c * HQ * D:(c + 1) * HQ * D],
            wk[:, c * D:(c + 1) * D],
            wv[:, c * D:(c + 1) * D],
        ], axis=1))
        woc = np.ascontiguousarray(wo[c * HQ * D:(c + 1) * HQ * D, :])
        in_maps.append({
            "xt": xt_m, "wqkv": cvt(wqkv_c), "woc": cvt(woc),
            "pmat": pmat_m, "tabs": tabs_m, "masks": masks_m,
        })
    return in_maps


_NC = None


def get_nc():
    global _NC
    if _NC is None:
        _NC = build_nc()
    return _NC


def kernel(x, cos, sin, wq, wk, wv, wo, q_norm_w, k_norm_w):
    nc = get_nc()
    in_maps = prep_inputs(x, cos, sin, wq, wk, wv, wo, q_norm_w, k_norm_w)
    res = run_bass_kernel_spmd(nc, in_maps, core_ids=list(range(NCORES)))
    acc = np.zeros((T, HID), dtype=np.float32)
    for c in range(NCORES):
        acc += res.results[c]["out"].astype(np.float32)
    return acc.reshape(B, S, HID)
